# revision 10
# baseline (speedup 1.0000x reference)
"""Bilateral filter (7x7, sigma_color=0.1) Trainium2 Bass kernel.

Strategy:
  - Host: zero-pad image, cast to fp16, shard 4(H) x 2(W) across 8 cores,
    pre-expand each core's shard into 7 pre-shifted "strip stacks":
      partition p = jy*18 + r  (7 row-shift strips x 18 rows = 126 partitions)
      ST[b, o, p, c*xw+x] = Ipad[c, y0 + jy + r, o + x]   (x-shift o baked in)
      CC[b, p, c*xw+x]    = Ipad[c, y0 + 3 + r, 3 + x]    (center, replicated)
  - Device per (block b, x-shift o in 0..6), elementwise fp16 (DVE 2x mode),
    emitted with a one-group software-pipeline skew (DMA / diff+sq / rest):
      diff   = ST[b,o] - CC      (DVE, one [126, 3*xw] unit-stride op)
      sq     = diff^2            (ACT Square, one [126, 3*xw] op)
      d1     = sq0 + sq1         (DVE)
      D      = d1 + sq2          (DVE)
      F      = exp(-50*D + bias_p) (ACT; bias_p = ln(norm_color*g[jy,o]) per strip)
      V_c    = F * ST_c          (DVE x3) -> packed tile vf = [V0 V1 V2 F]
      accumulate over (jy, o) on TensorE: 5 matmuls N=512,
        psum[18, 2560] += collapse[126,18].T @ vf   (fp32 accumulation)
  - Finalize per 7-block group (emitted inline): evacuate psum into strip-
    gathered [126, xw] tiles, reciprocal(den) on DVE, num*rec on GPSIMD.
  - Duplicate LDWEIGHTS of the stationary collapse matrix are pruned from the
    BIR before walrus compiles it (PE weights persist across matmuls).
"""

import math

import numpy as np

import concourse.bass as bass
import concourse.bacc as bacc
import concourse.mybir as mybir
from concourse.tile import TileContext

F16 = np.float16
F32 = np.float32

# problem constants
H, W, C = 720, 1280, 3
K = 7
PAD = 3
SIGMA_COLOR = 0.1
NORM_COLOR = 1.0 / (2.0 * math.pi * SIGMA_COLOR**2)
EXP_SCALE = -1.0 / (2.0 * SIGMA_COLOR**2)  # -50.0

# sharding / tiling constants
HSH, WSH = 4, 2          # core grid (8 cores)
RB = 18                  # output rows per block
JY = 7                   # row-shift strips
P = JY * RB              # 126 partitions used
XW = W // WSH            # 640 output cols per core
N_CORES = 8
MMN = 512                # matmul free-dim max (one PSUM bank)


def _alu(name):
    return getattr(mybir.AluOpType, name)


def build_nc(nb: int, xw: int = XW):
    """Build the Bass program for one core processing nb blocks of RB rows x xw cols."""
    dt = mybir.dt
    nc = bacc.Bacc("TRN2", debug=False)

    ST = nc.dram_tensor("ST", [nb, K, P, C * xw], dt.float16, kind="ExternalInput")
    CC = nc.dram_tensor("CC", [nb, P, C * xw], dt.float16, kind="ExternalInput")
    BI = nc.dram_tensor("BI", [128, 8], dt.float32, kind="ExternalInput")
    CL = nc.dram_tensor("CL", [P, RB], dt.float16, kind="ExternalInput")
    OUT = nc.dram_tensor("OUT", [C, nb * RB, xw], dt.float32, kind="ExternalOutput")

    fw = (C + 1) * xw  # packed vf width (V0 V1 V2 F)
    assert fw % MMN == 0
    n_mm = fw // MMN
    half = fw // 2

    n_grp = (nb + 6) // 7
    grp_rows = [min(7, nb - 7 * g) * RB for g in range(n_grp)]
    total = nb * K

    with TileContext(nc) as tc:
        with (
            tc.tile_pool(name="singles", bufs=1) as psingle,
            tc.tile_pool(name="stack", bufs=4) as pstack,
            tc.tile_pool(name="cstack", bufs=2) as pcstack,
            tc.tile_pool(name="work", bufs=4) as pwork,
            tc.tile_pool(name="psum", bufs=1, space="PSUM") as ppsum,
            tc.tile_pool(name="stage", bufs=2) as pstage,
            tc.tile_pool(name="gather", bufs=1) as pgather,
            tc.tile_pool(name="fin", bufs=2) as pfin,
        ):
            bi = psingle.tile([128, 8], dt.float32, tag="bias")
            nc.sync.dma_start(bi[:, :], BI[:, :])
            cl = psingle.tile([P, RB], dt.float16, tag="coll")
            nc.sync.dma_start(cl[:, :], CL[:, :])

            gat = {}
            for f_i in range(4):  # 0..2 = num channels, 3 = den
                for g in range(n_grp):
                    gat[(f_i, g)] = pgather.tile(
                        [126, xw], dt.float32, tag=f"gat{f_i}_{g}", name=f"gat{f_i}_{g}"
                    )

            st_t, df_t, sq_t, cc_t, pp_t = {}, {}, {}, {}, {}

            def stage_a(gi):
                b, o = divmod(gi, K)
                if o == 0:
                    cc = pcstack.tile([P, C * xw], dt.float16, tag="cc", name="cc")
                    nc.sync.dma_start(cc[:, :], CC[b])
                    cc_t[b] = cc
                st = pstack.tile([P, C * xw], dt.float16, tag="st", name="st")
                nc.sync.dma_start(st[:, :], ST[b, o])
                st_t[gi] = st

            def stage_b(gi):
                b, o = divmod(gi, K)
                df = pwork.tile([P, C * xw], dt.float16, tag="df", name="df")
                nc.vector.tensor_tensor(
                    df[:, :], st_t[gi][:, :], cc_t[b][:, :], _alu("subtract")
                )
                df_t[gi] = df
                sq = pwork.tile([P, C * xw], dt.float16, tag="sq", name="sq")
                nc.scalar.activation(
                    sq[:, :], df[:, :], mybir.ActivationFunctionType.Square
                )
                sq_t[gi] = sq

            def finalize(g):
                rg = grp_rows[g]
                rec = pfin.tile([126, xw], dt.float32, tag="rec", name="rec")
                nc.vector.reciprocal(rec[0:rg, :], gat[(3, g)][0:rg, :])
                for c in range(C):
                    ot = pfin.tile([126, xw], dt.float32, tag="ot", name="ot")
                    nc.gpsimd.tensor_tensor(
                        ot[0:rg, :], gat[(c, g)][0:rg, :], rec[0:rg, :], _alu("mult")
                    )
                    nc.sync.dma_start(OUT[c, g * 126 : g * 126 + rg, :], ot[0:rg, :])

            def stage_c(gi):
                b, o = divmod(gi, K)
                if o == 0:
                    pp_t[b] = ppsum.tile([RB, fw], dt.float32, tag="pp", name="pp")
                pp = pp_t[b]
                st, sq = st_t[gi], sq_t[gi]

                d1 = pwork.tile([P, xw], dt.float16, tag="d1", name="d1")
                nc.vector.tensor_tensor(
                    d1[:], sq[:, 0:xw], sq[:, xw : 2 * xw], _alu("add")
                )
                d2 = pwork.tile([P, xw], dt.float16, tag="d2", name="d2")
                nc.vector.tensor_tensor(
                    d2[:], d1[:], sq[:, 2 * xw : 3 * xw], _alu("add")
                )

                vf = pwork.tile([P, fw], dt.float16, tag="vf", name="vf")
                nc.scalar.activation(
                    vf[:, C * xw : fw],
                    d2[:],
                    mybir.ActivationFunctionType.Exp,
                    bias=bi[0:P, o : o + 1],
                    scale=float(EXP_SCALE),
                )
                for c in range(C):
                    nc.vector.tensor_tensor(
                        vf[:, c * xw : (c + 1) * xw],
                        vf[:, C * xw : fw],
                        st[:, c * xw : (c + 1) * xw],
                        _alu("mult"),
                    )

                st_, sp_ = (o == 0), (o == K - 1)
                for m in range(n_mm):
                    nc.tensor.matmul(
                        pp[:, m * MMN : (m + 1) * MMN],
                        cl[:, :],
                        vf[:, m * MMN : (m + 1) * MMN],
                        start=st_,
                        stop=sp_,
                    )

                del st_t[gi], df_t[gi], sq_t[gi]
                if o == K - 1:
                    g, idx = b // 7, b % 7
                    stga = pstage.tile([RB, half], dt.float32, tag="stga", name="stga")
                    nc.scalar.copy(stga[:], pp[:, 0:half])
                    stgb = pstage.tile([RB, half], dt.float32, tag="stgb", name="stgb")
                    nc.scalar.copy(stgb[:], pp[:, half:fw])
                    rows = slice(idx * RB, (idx + 1) * RB)
                    nc.sync.dma_start(gat[(0, g)][rows, :], stga[:, 0:xw])
                    nc.sync.dma_start(gat[(1, g)][rows, :], stga[:, xw : 2 * xw])
                    nc.sync.dma_start(gat[(2, g)][rows, :], stgb[:, 0:xw])
                    nc.sync.dma_start(gat[(3, g)][rows, :], stgb[:, xw : 2 * xw])
                    del pp_t[b], cc_t[b]
                    if b == 7 * g + 6 or b == nb - 1:
                        finalize(g)

            # software-pipelined emission: DMA two groups ahead, diff/sq one ahead
            stage_a(0)
            if total > 1:
                stage_a(1)
            stage_b(0)
            for gi in range(total):
                if gi + 2 < total:
                    stage_a(gi + 2)
                if gi + 1 < total:
                    stage_b(gi + 1)
                stage_c(gi)

    nc.compile()
    return nc


def host_prepare(I: np.ndarray, gw49: np.ndarray):
    """I: (1, C, Him, Wim) fp32. Returns in_maps for 8 cores + assembly info."""
    _, c_, him, wim = I.shape
    assert c_ == C
    nb = him // (HSH * RB)
    xw = wim // WSH
    rs = nb * RB  # rows per core

    Ip = np.zeros((C, him + 2 * PAD, wim + 2 * PAD), dtype=F32)
    Ip[:, PAD : PAD + him, PAD : PAD + wim] = I[0]
    Ib = Ip.astype(F16)

    # bias + collapse (shared across cores)
    bias = np.zeros((128, 8), dtype=F32)
    gw7 = gw49.reshape(K, K).astype(np.float64)
    for p in range(P):
        jy = p // RB
        bias[p, :K] = np.log(NORM_COLOR * gw7[jy, :]).astype(F32)
    coll = np.zeros((P, RB), dtype=F16)
    for p in range(P):
        coll[p, p % RB] = 1.0

    in_maps = []
    for i in range(N_CORES):
        hi, wi = i // WSH, i % WSH
        sh = Ib[:, rs * hi : rs * hi + rs + 2 * PAD, xw * wi : xw * wi + xw + 2 * PAD]
        s0, s1, s2 = sh.strides
        # ST[b, o, (jy, r), c, x] = sh[c, b*RB + jy + r, o + x]
        w1 = np.lib.stride_tricks.as_strided(
            sh,
            shape=(C, nb, K, JY, RB, xw),
            strides=(s0, RB * s1, s2, s1, s1, s2),
        )
        STa = np.ascontiguousarray(w1.transpose(1, 2, 3, 4, 0, 5)).reshape(
            nb, K, P, C * xw
        )
        shc = sh[:, PAD:, PAD:]
        w3 = np.lib.stride_tricks.as_strided(
            shc, shape=(C, nb, JY, RB, xw), strides=(s0, RB * s1, 0, s1, s2)
        )
        CCa = np.ascontiguousarray(w3.transpose(1, 2, 3, 0, 4)).reshape(
            nb, P, C * xw
        )
        in_maps.append({"ST": STa, "CC": CCa, "BI": bias, "CL": coll})
    return in_maps, nb, xw, rs


def assemble(results, him, wim, rs, xw):
    out = np.empty((1, C, him, wim), dtype=F32)
    for i in range(N_CORES):
        hi, wi = i // WSH, i % WSH
        out[0, :, rs * hi : rs * hi + rs, xw * wi : xw * wi + xw] = results[i]["OUT"]
    return out


def _numpy_fallback(I, g):
    """Exact reference computation on host (used only if g is not spatially constant)."""
    n, c, h, w = I.shape
    Ipad = np.zeros((n, c, h + 2 * PAD, w + 2 * PAD), dtype=np.float64)
    Ipad[:, :, PAD : PAD + h, PAD : PAD + w] = I
    num = np.zeros((n, c, h, w), dtype=np.float64)
    den = np.zeros((n, h, w), dtype=np.float64)
    g64 = g.astype(np.float64)
    for j in range(K * K):
        dy, dx = j // K, j % K
        S = Ipad[:, :, dy : dy + h, dx : dx + w]
        D = ((S - I.astype(np.float64)) ** 2).sum(axis=1)
        wgt = np.exp(EXP_SCALE * D) * NORM_COLOR * g64[:, j]
        num += wgt[:, None] * S
        den += wgt
    return (num / den[:, None]).astype(F32)


_CACHE = {}
TRACE = False
LAST_EXEC_NS = None
_LDW_PATCHED = False


def _enable_ldw_prune():
    """Drop duplicate LDWEIGHTS of the same stationary lhsT from the BIR before
    walrus compiles it. PE weights persist across matmuls; only loads that carry
    sync conditions (or follow a different weight tensor) are kept."""
    global _LDW_PATCHED
    if _LDW_PATCHED:
        return
    import json as _json
    import concourse.bass_utils as _bu

    _orig = _bu.compile_bir_kernel

    def _prune(bir_json):
        js = _json.loads(bir_json)
        for fn in js.get("functions", []):
            for blk in fn.get("blocks", []):
                insts = blk.get("instructions", [])
                out = []
                last_ldw = None
                for inst in insts:
                    if inst.get("opcode") == "Ldweights":
                        si = inst.get("sync_info") or {}
                        key = _json.dumps(inst.get("ins"), sort_keys=True)
                        if (
                            last_ldw == key
                            and not si.get("on_wait")
                            and not si.get("on_update")
                        ):
                            continue  # duplicate load of identical weights
                        last_ldw = key
                    out.append(inst)
                blk["instructions"] = out
        return _json.dumps(js).encode()

    def _patched(bir_json, tmpdir, neff_name="file.neff"):
        try:
            bir_json = _prune(bir_json)
        except Exception:
            pass
        return _orig(bir_json, tmpdir, neff_name=neff_name)

    _bu.compile_bir_kernel = _patched
    try:
        import concourse.bass2jax as _b2j

        if getattr(_b2j, "compile_bir_kernel", None) is not None:
            _b2j.compile_bir_kernel = _patched
    except Exception:
        pass
    _LDW_PATCHED = True


def kernel(I: np.ndarray, g: np.ndarray) -> np.ndarray:
    global LAST_EXEC_NS
    I = np.asarray(I, dtype=F32)
    g = np.asarray(g)

    gw49 = np.asarray(g[0, :, 0, 0], dtype=F32)
    if not np.array_equal(
        np.asarray(g), np.broadcast_to(np.asarray(g)[:, :, :1, :1], g.shape)
    ):
        return _numpy_fallback(I, g)

    from concourse.bass_utils import run_bass_kernel_spmd

    import os as _os
    if _os.environ.get("BASS_LDW_PRUNE", "1") == "1":
        _enable_ldw_prune()

    in_maps, nb, xw, rs = host_prepare(I, gw49)
    key = (nb, xw)
    if key not in _CACHE:
        _CACHE[key] = build_nc(nb, xw)
    nc = _CACHE[key]
    res = run_bass_kernel_spmd(
        nc, in_maps, core_ids=list(range(N_CORES)), trace=TRACE
    )
    LAST_EXEC_NS = res.exec_time_ns
    return assemble(res.results, I.shape[2], I.shape[3], rs, xw)


if __name__ == "__main__":
    # tiny smoke test in CoreSim: 1 core, small image
    import concourse.bass_interp as bass_interp

    rng = np.random.default_rng(0)
    him, wim = HSH * RB * 2, W  # 2 blocks per core
    I = rng.random((1, C, him, wim), dtype=F32)
    gw49 = np.exp(
        -(np.add.outer(np.arange(-3.0, 4) ** 2, np.arange(-3.0, 4) ** 2)) / 50.0
    ).reshape(-1) * (2 * math.pi * 25.0)
    g = np.tile(gw49.reshape(1, K * K, 1, 1), (1, 1, him, wim)).astype(F32)

    in_maps, nb, xw, rs = host_prepare(I, gw49.astype(F32))
    nc = build_nc(nb, xw)
    sim = bass_interp.CoreSim(nc)
    for k, v in in_maps[0].items():
        sim.tensor(k)[:] = v
    sim.simulate()
    got = np.array(sim.tensor("OUT"))

    exp_full = _numpy_fallback(I, g)
    exp0 = exp_full[0, :, 0:rs, 0:xw]
    err = np.abs(got - exp0)
    print("sim err max:", err.max(), "rel:", err.max() / np.abs(exp0).max())


# revision 12
# speedup vs baseline: 1.0247x; 1.0247x over previous
"""Bilateral filter (7x7, sigma_color=0.1) Trainium2 Bass kernel.

Strategy:
  - Host: zero-pad image, cast to fp16, shard 4(H) x 2(W) across 8 cores,
    pre-expand each core's shard into 7 pre-shifted "strip stacks":
      partition p = jy*18 + r  (7 row-shift strips x 18 rows = 126 partitions)
      ST[b, o, p, c*xw+x] = Ipad[c, y0 + jy + r, o + x]   (x-shift o baked in)
      CC[b, p, c*xw+x]    = Ipad[c, y0 + 3 + r, 3 + x]    (center, replicated)
  - Device per (block b, x-shift o in 0..6), elementwise fp16 (DVE 2x mode):
      diff   = ST[b,o] - CC      (DVE, one [126, 3*xw] unit-stride op)
      sq     = diff^2            (ACT Square, one [126, 3*xw] op)
      d1     = sq0 + sq1         (DVE)
      D      = d1 + sq2          (DVE)
      F      = exp(-50*D + bias_p) (ACT; bias_p = ln(norm_color*g[jy,o]) per strip)
      V_c    = F * ST_c          (DVE x3) -> packed tile vf = [V0 V1 V2 F]
      accumulate over (jy, o) on TensorE: 5 matmuls N=512,
        psum[18, 2560] += collapse[126,18].T @ vf   (fp32 accumulation)
  - Finalize per 7-block group (emitted inline): evacuate psum into strip-
    gathered [126, xw] tiles, reciprocal(den) on DVE, num*rec on GPSIMD.
  - Duplicate LDWEIGHTS of the stationary collapse matrix are pruned from the
    BIR before walrus compiles it (PE weights persist across matmuls).
"""

import math

import numpy as np

import concourse.bass as bass
import concourse.bacc as bacc
import concourse.mybir as mybir
from concourse.tile import TileContext

F16 = np.float16
F32 = np.float32

# problem constants
H, W, C = 720, 1280, 3
K = 7
PAD = 3
SIGMA_COLOR = 0.1
NORM_COLOR = 1.0 / (2.0 * math.pi * SIGMA_COLOR**2)
EXP_SCALE = -1.0 / (2.0 * SIGMA_COLOR**2)  # -50.0

# sharding / tiling constants
HSH, WSH = 4, 2          # core grid (8 cores)
RB = 18                  # output rows per block
JY = 7                   # row-shift strips
P = JY * RB              # 126 partitions used
XW = W // WSH            # 640 output cols per core
N_CORES = 8
MMN = 512                # matmul free-dim max (one PSUM bank)


def _alu(name):
    return getattr(mybir.AluOpType, name)


def build_nc(nb: int, xw: int = XW):
    """Build the Bass program for one core processing nb blocks of RB rows x xw cols."""
    dt = mybir.dt
    nc = bacc.Bacc("TRN2", debug=False)

    ST = nc.dram_tensor("ST", [nb, K, P, C * xw], dt.float16, kind="ExternalInput")
    CC = nc.dram_tensor("CC", [nb, P, C * xw], dt.float16, kind="ExternalInput")
    BI = nc.dram_tensor("BI", [128, 8], dt.float32, kind="ExternalInput")
    CL = nc.dram_tensor("CL", [P, RB], dt.float16, kind="ExternalInput")
    OUT = nc.dram_tensor("OUT", [C, nb * RB, xw], dt.float32, kind="ExternalOutput")

    fw = (C + 1) * xw  # packed vf width (V0 V1 V2 F)
    assert fw % MMN == 0
    n_mm = fw // MMN
    half = fw // 2

    n_grp = (nb + 6) // 7
    grp_rows = [min(7, nb - 7 * g) * RB for g in range(n_grp)]
    total = nb * K

    with TileContext(nc) as tc:
        with (
            tc.tile_pool(name="singles", bufs=1) as psingle,
            tc.tile_pool(name="stack", bufs=4) as pstack,
            tc.tile_pool(name="cstack", bufs=2) as pcstack,
            tc.tile_pool(name="work", bufs=4) as pwork,
            tc.tile_pool(name="psum", bufs=1, space="PSUM") as ppsum,
            tc.tile_pool(name="stage", bufs=2) as pstage,
            tc.tile_pool(name="gather", bufs=1) as pgather,
            tc.tile_pool(name="fin", bufs=2) as pfin,
        ):
            bi = psingle.tile([128, 8], dt.float32, tag="bias")
            nc.sync.dma_start(bi[:, :], BI[:, :])
            cl = psingle.tile([P, RB], dt.float16, tag="coll")
            nc.sync.dma_start(cl[:, :], CL[:, :])

            gat = {}
            for f_i in range(4):  # 0..2 = num channels, 3 = den
                for g in range(n_grp):
                    gat[(f_i, g)] = pgather.tile(
                        [126, xw], dt.float32, tag=f"gat{f_i}_{g}", name=f"gat{f_i}_{g}"
                    )

            st_t, df_t, sq_t, cc_t, pp_t = {}, {}, {}, {}, {}

            def stage_a(gi):
                b, o = divmod(gi, K)
                if o == 0:
                    cc = pcstack.tile([P, C * xw], dt.float16, tag="cc", name="cc")
                    nc.sync.dma_start(cc[:, :], CC[b])
                    cc_t[b] = cc
                st = pstack.tile([P, C * xw], dt.float16, tag="st", name="st")
                nc.sync.dma_start(st[:, :], ST[b, o])
                st_t[gi] = st

            def stage_b(gi):
                b, o = divmod(gi, K)
                df = pwork.tile([P, C * xw], dt.float16, tag="df", name="df")
                nc.vector.tensor_tensor(
                    df[:, :], st_t[gi][:, :], cc_t[b][:, :], _alu("subtract")
                )
                df_t[gi] = df
                sq = pwork.tile([P, C * xw], dt.float16, tag="sq", name="sq")
                nc.scalar.activation(
                    sq[:, :], df[:, :], mybir.ActivationFunctionType.Square
                )
                sq_t[gi] = sq

            def finalize(g):
                rg = grp_rows[g]
                rec = pfin.tile([126, xw], dt.float32, tag="rec", name="rec")
                nc.vector.reciprocal(rec[0:rg, :], gat[(3, g)][0:rg, :])
                for c in range(C):
                    ot = pfin.tile([126, xw], dt.float32, tag="ot", name="ot")
                    nc.gpsimd.tensor_tensor(
                        ot[0:rg, :], gat[(c, g)][0:rg, :], rec[0:rg, :], _alu("mult")
                    )
                    nc.sync.dma_start(OUT[c, g * 126 : g * 126 + rg, :], ot[0:rg, :])

            def stage_c(gi):
                b, o = divmod(gi, K)
                if o == 0:
                    pp_t[b] = ppsum.tile([RB, fw], dt.float32, tag="pp", name="pp")
                pp = pp_t[b]
                st, sq = st_t[gi], sq_t[gi]

                d1 = pwork.tile([P, xw], dt.float16, tag="d1", name="d1")
                nc.vector.tensor_tensor(
                    d1[:], sq[:, 0:xw], sq[:, xw : 2 * xw], _alu("add")
                )
                d2 = pwork.tile([P, xw], dt.float16, tag="d2", name="d2")
                nc.vector.tensor_tensor(
                    d2[:], d1[:], sq[:, 2 * xw : 3 * xw], _alu("add")
                )

                vf = pwork.tile([P, fw], dt.float16, tag="vf", name="vf")
                nc.scalar.activation(
                    vf[:, C * xw : fw],
                    d2[:],
                    mybir.ActivationFunctionType.Exp,
                    bias=bi[0:P, o : o + 1],
                    scale=float(EXP_SCALE),
                )
                for c in range(C):
                    nc.vector.tensor_tensor(
                        vf[:, c * xw : (c + 1) * xw],
                        vf[:, C * xw : fw],
                        st[:, c * xw : (c + 1) * xw],
                        _alu("mult"),
                    )

                st_, sp_ = (o == 0), (o == K - 1)
                for m in range(n_mm):
                    nc.tensor.matmul(
                        pp[:, m * MMN : (m + 1) * MMN],
                        cl[:, :],
                        vf[:, m * MMN : (m + 1) * MMN],
                        start=st_,
                        stop=sp_,
                    )

                del st_t[gi], df_t[gi], sq_t[gi]
                if o == K - 1:
                    g, idx = b // 7, b % 7
                    stga = pstage.tile([RB, half], dt.float32, tag="stga", name="stga")
                    nc.scalar.copy(stga[:], pp[:, 0:half])
                    stgb = pstage.tile([RB, half], dt.float32, tag="stgb", name="stgb")
                    nc.scalar.copy(stgb[:], pp[:, half:fw])
                    rows = slice(idx * RB, (idx + 1) * RB)
                    nc.sync.dma_start(gat[(0, g)][rows, :], stga[:, 0:xw])
                    nc.sync.dma_start(gat[(1, g)][rows, :], stga[:, xw : 2 * xw])
                    nc.sync.dma_start(gat[(2, g)][rows, :], stgb[:, 0:xw])
                    nc.sync.dma_start(gat[(3, g)][rows, :], stgb[:, xw : 2 * xw])
                    del pp_t[b], cc_t[b]
                    if b == 7 * g + 6 or b == nb - 1:
                        finalize(g)

            # plain in-order emission (Tile's scheduler handles overlap;
            # measured better than an explicit software-pipeline skew)
            for gi in range(total):
                stage_a(gi)
                stage_b(gi)
                stage_c(gi)

    nc.compile()
    return nc


def host_prepare(I: np.ndarray, gw49: np.ndarray):
    """I: (1, C, Him, Wim) fp32. Returns in_maps for 8 cores + assembly info."""
    _, c_, him, wim = I.shape
    assert c_ == C
    nb = him // (HSH * RB)
    xw = wim // WSH
    rs = nb * RB  # rows per core

    Ip = np.zeros((C, him + 2 * PAD, wim + 2 * PAD), dtype=F32)
    Ip[:, PAD : PAD + him, PAD : PAD + wim] = I[0]
    Ib = Ip.astype(F16)

    # bias + collapse (shared across cores)
    bias = np.zeros((128, 8), dtype=F32)
    gw7 = gw49.reshape(K, K).astype(np.float64)
    for p in range(P):
        jy = p // RB
        bias[p, :K] = np.log(NORM_COLOR * gw7[jy, :]).astype(F32)
    coll = np.zeros((P, RB), dtype=F16)
    for p in range(P):
        coll[p, p % RB] = 1.0

    in_maps = []
    for i in range(N_CORES):
        hi, wi = i // WSH, i % WSH
        sh = Ib[:, rs * hi : rs * hi + rs + 2 * PAD, xw * wi : xw * wi + xw + 2 * PAD]
        s0, s1, s2 = sh.strides
        # ST[b, o, (jy, r), c, x] = sh[c, b*RB + jy + r, o + x]
        w1 = np.lib.stride_tricks.as_strided(
            sh,
            shape=(C, nb, K, JY, RB, xw),
            strides=(s0, RB * s1, s2, s1, s1, s2),
        )
        STa = np.ascontiguousarray(w1.transpose(1, 2, 3, 4, 0, 5)).reshape(
            nb, K, P, C * xw
        )
        shc = sh[:, PAD:, PAD:]
        w3 = np.lib.stride_tricks.as_strided(
            shc, shape=(C, nb, JY, RB, xw), strides=(s0, RB * s1, 0, s1, s2)
        )
        CCa = np.ascontiguousarray(w3.transpose(1, 2, 3, 0, 4)).reshape(
            nb, P, C * xw
        )
        in_maps.append({"ST": STa, "CC": CCa, "BI": bias, "CL": coll})
    return in_maps, nb, xw, rs


def assemble(results, him, wim, rs, xw):
    out = np.empty((1, C, him, wim), dtype=F32)
    for i in range(N_CORES):
        hi, wi = i // WSH, i % WSH
        out[0, :, rs * hi : rs * hi + rs, xw * wi : xw * wi + xw] = results[i]["OUT"]
    return out


def _numpy_fallback(I, g):
    """Exact reference computation on host (used only if g is not spatially constant)."""
    n, c, h, w = I.shape
    Ipad = np.zeros((n, c, h + 2 * PAD, w + 2 * PAD), dtype=np.float64)
    Ipad[:, :, PAD : PAD + h, PAD : PAD + w] = I
    num = np.zeros((n, c, h, w), dtype=np.float64)
    den = np.zeros((n, h, w), dtype=np.float64)
    g64 = g.astype(np.float64)
    for j in range(K * K):
        dy, dx = j // K, j % K
        S = Ipad[:, :, dy : dy + h, dx : dx + w]
        D = ((S - I.astype(np.float64)) ** 2).sum(axis=1)
        wgt = np.exp(EXP_SCALE * D) * NORM_COLOR * g64[:, j]
        num += wgt[:, None] * S
        den += wgt
    return (num / den[:, None]).astype(F32)


_CACHE = {}
TRACE = False
LAST_EXEC_NS = None
_LDW_PATCHED = False


def _enable_ldw_prune():
    """Drop duplicate LDWEIGHTS of the same stationary lhsT from the BIR before
    walrus compiles it. PE weights persist across matmuls; only loads that carry
    sync conditions (or follow a different weight tensor) are kept."""
    global _LDW_PATCHED
    if _LDW_PATCHED:
        return
    import json as _json
    import concourse.bass_utils as _bu

    _orig = _bu.compile_bir_kernel

    def _prune(bir_json):
        js = _json.loads(bir_json)
        for fn in js.get("functions", []):
            for blk in fn.get("blocks", []):
                insts = blk.get("instructions", [])
                out = []
                last_ldw = None
                for inst in insts:
                    if inst.get("opcode") == "Ldweights":
                        si = inst.get("sync_info") or {}
                        key = _json.dumps(inst.get("ins"), sort_keys=True)
                        if (
                            last_ldw == key
                            and not si.get("on_wait")
                            and not si.get("on_update")
                        ):
                            continue  # duplicate load of identical weights
                        last_ldw = key
                    out.append(inst)
                blk["instructions"] = out
        return _json.dumps(js).encode()

    def _patched(bir_json, tmpdir, neff_name="file.neff"):
        try:
            bir_json = _prune(bir_json)
        except Exception:
            pass
        return _orig(bir_json, tmpdir, neff_name=neff_name)

    _bu.compile_bir_kernel = _patched
    try:
        import concourse.bass2jax as _b2j

        if getattr(_b2j, "compile_bir_kernel", None) is not None:
            _b2j.compile_bir_kernel = _patched
    except Exception:
        pass
    _LDW_PATCHED = True


def kernel(I: np.ndarray, g: np.ndarray) -> np.ndarray:
    global LAST_EXEC_NS
    I = np.asarray(I, dtype=F32)
    g = np.asarray(g)

    gw49 = np.asarray(g[0, :, 0, 0], dtype=F32)
    if not np.array_equal(
        np.asarray(g), np.broadcast_to(np.asarray(g)[:, :, :1, :1], g.shape)
    ):
        return _numpy_fallback(I, g)

    from concourse.bass_utils import run_bass_kernel_spmd

    import os as _os
    if _os.environ.get("BASS_LDW_PRUNE", "1") == "1":
        _enable_ldw_prune()

    in_maps, nb, xw, rs = host_prepare(I, gw49)
    key = (nb, xw)
    if key not in _CACHE:
        _CACHE[key] = build_nc(nb, xw)
    nc = _CACHE[key]
    res = run_bass_kernel_spmd(
        nc, in_maps, core_ids=list(range(N_CORES)), trace=TRACE
    )
    LAST_EXEC_NS = res.exec_time_ns
    return assemble(res.results, I.shape[2], I.shape[3], rs, xw)


if __name__ == "__main__":
    # tiny smoke test in CoreSim: 1 core, small image
    import concourse.bass_interp as bass_interp

    rng = np.random.default_rng(0)
    him, wim = HSH * RB * 2, W  # 2 blocks per core
    I = rng.random((1, C, him, wim), dtype=F32)
    gw49 = np.exp(
        -(np.add.outer(np.arange(-3.0, 4) ** 2, np.arange(-3.0, 4) ** 2)) / 50.0
    ).reshape(-1) * (2 * math.pi * 25.0)
    g = np.tile(gw49.reshape(1, K * K, 1, 1), (1, 1, him, wim)).astype(F32)

    in_maps, nb, xw, rs = host_prepare(I, gw49.astype(F32))
    nc = build_nc(nb, xw)
    sim = bass_interp.CoreSim(nc)
    for k, v in in_maps[0].items():
        sim.tensor(k)[:] = v
    sim.simulate()
    got = np.array(sim.tensor("OUT"))

    exp_full = _numpy_fallback(I, g)
    exp0 = exp_full[0, :, 0:rs, 0:xw]
    err = np.abs(got - exp0)
    print("sim err max:", err.max(), "rel:", err.max() / np.abs(exp0).max())
